# revision 1
# baseline (speedup 1.0000x reference)
"""MultiResCNN output layer (per-label attention) on 8 TRN2 NeuronCores.

Math (reference):
    scores = einsum('yd,bld->byl', U, x)
    alpha  = softmax(scores, axis=l)
    m      = einsum('byl,bld->byd', alpha, x)
    y      = einsum('yd,byd->by', fw, m) + bias

Sharding: 2 label-shards x 4 batch-shards over 8 cores. Each core handles
BC=2 batches x YC=4480 (padded) labels, fully independently (no collectives).

Per-core kernel (all matmuls bf16, fp32 PSUM accumulation):
  MM1: scoresT[l, y] = sum_d xT[d, l] * UT[d, y]   (l on partitions, y free)
  exp: eT = exp(scoresT)  -- ACT engine, PSUM -> SBUF bf16, no max-subtract
       (scores are bounded: |score| <~ 2.5, exp can't overflow)
  MM2: m[y, d] = sum_l eT[l, y] * x[l, d]  -- both operands natural layout
       (padded l rows of x are zero, so they never pollute m)
  Z:   eSum[l, y-group] accumulated on DVE across the 20 l-chunks, then one
       (128x128)x(128x1) ones-matmul per label subtile gives the partition
       sum; padded l rows contribute exp(0)=1 each, corrected exactly by
       subtracting NPAD_L.
  fin: y[y] = (sum_d fw[y,d]*m[y,d]) / Z[y] + bias[y]   (DVE)

The softmax normalization is algebraically deferred to the final scalar
divide: alpha = e/Z  =>  y = (fw . (e @ x)) / Z + bias.

Post-passes on the BIR: _split_sync_waits (this walrus accepts only one sync
wait per instruction) and _thin_engine_clock_updates (drop the ~2400 PE
semaphore increments no consumer ever waits on; each costs ~26 ns of PE
sequencer time).
"""

import contextlib
import os
import sys

import numpy as np

for _p in ("/opt/trn_rl_repo", "/root/.axon_site/_ro/trn_rl_repo"):
    if os.path.isdir(_p) and _p not in sys.path:
        sys.path.insert(0, _p)

import ml_dtypes  # noqa: E402

import concourse.bass as bass  # noqa: E402
import concourse.mybir as mybir  # noqa: E402
import concourse.tile as tile  # noqa: E402
from concourse.bass_utils import run_bass_kernel_spmd  # noqa: E402

BF16 = mybir.dt.bfloat16
F32 = mybir.dt.float32
nbf16 = ml_dtypes.bfloat16

B, L, D, Y = 8, 2500, 512, 8921
NCORES = 8
YSHARDS, BSHARDS = 2, 4
BC = B // BSHARDS  # batches per core
YC = 4480  # padded labels per y-shard (2*4480 = 8960 >= 8921)
LPAD = 2560  # padded seq len (20 chunks of 128)
LC = LPAD // 128  # 20 l-chunks
DC = D // 128  # 4 d-chunks
YG = 512  # max y-group (MM1 psum free size)
GROUPS = [(off, min(YG, YC - off)) for off in range(0, YC, YG)]  # 8x512 + 1x384
JTOT = YC // 128  # 35 label subtiles per core
NPAD_L = LPAD - L  # 60: every padded l row contributes exp(0)=1 to eSum


def _split_sync_waits(nc, max_waits=1):
    """This container's walrus rejects >1 sync wait per instruction ("Too many
    sync wait commands"); move excess waits onto chained same-engine NOPs."""
    for fn in nc.m.functions:
        for bb in fn.blocks:
            new_list = []
            for ins in bb.instructions:
                si = ins.sync_info
                if si is not None and si.on_wait is not None and len(si.on_wait) > max_waits:
                    waits = list(si.on_wait)
                    head, tail = waits[:-max_waits], waits[-max_waits:]
                    idx = 0
                    while head:
                        chunk, head = head[:max_waits], head[max_waits:]
                        nop = mybir.InstNoOp(name=f"{ins.name}-ws{idx}", ins=[], outs=[])
                        nop.engine = ins.engine
                        nop.sync_info = mybir.SyncInfo(on_wait=chunk, on_update=[])
                        new_list.append(nop)
                        idx += 1
                    ins.sync_info = mybir.SyncInfo(
                        on_wait=tail,
                        on_update=list(si.on_update) if si.on_update else [],
                    )
                new_list.append(ins)
            bb.instructions[:] = new_list




def _thin_engine_clock_updates(nc, engine_names=("PE",)):
    """Drop per-instruction engine-clock semaphore increments whose tick value
    no wait ever references, renumbering the remaining waits. Each increment
    costs ~26 ns serialized on the engine; matmul-dense programs tick ~6x more
    often than consumers look. Only valid for linear (no control flow)
    programs; callers must not apply it to For_i benchmark builds."""
    all_insts = [
        ins
        for fn in nc.m.functions
        for bb in fn.blocks
        for ins in bb.instructions
    ]
    # semaphores and how they are updated / waited
    upd_engines = {}
    upd_modes = {}
    waited = {}
    for ins in all_insts:
        si = ins.sync_info
        if si is None:
            continue
        for u in si.on_update or []:
            upd_engines.setdefault(u.id, set()).add(str(ins.engine))
            upd_modes.setdefault(u.id, set()).add((u.update_mode, u.update_value))
        for w in si.on_wait or []:
            waited.setdefault(w.id, set())
            if w.wait_mode == "sem-ge-imm":
                waited[w.id].add(w.wait_value)
            else:
                waited[w.id].add("ALL")

    target_engines = {f"EngineType.{e}" for e in engine_names}
    for sem_id, engs in upd_engines.items():
        if len(engs) != 1 or not engs <= target_engines:
            continue
        if upd_modes[sem_id] != {("sem-inc", 1)}:
            continue
        wvals = waited.get(sem_id, set())
        if "ALL" in wvals:
            continue
        # tick numbering in program order
        seq = []
        tick = 0
        for ins in all_insts:
            si = ins.sync_info
            if si and si.on_update:
                for u in si.on_update:
                    if u.id == sem_id:
                        tick += 1
                        seq.append((ins, tick))
        total = tick
        keep = {t for t in wvals if isinstance(t, int)}
        keep.add(total)
        assert all(1 <= t <= total for t in keep), (sem_id, keep, total)
        kept_sorted = sorted(keep)
        rank = {t: i + 1 for i, t in enumerate(kept_sorted)}
        for ins, t in seq:
            if t in keep:
                continue
            si = ins.sync_info
            new_upd = [u for u in si.on_update if u.id != sem_id]
            ins.sync_info = mybir.SyncInfo(
                on_wait=list(si.on_wait or []), on_update=new_upd
            )
        for ins in all_insts:
            si = ins.sync_info
            if not (si and si.on_wait):
                continue
            if not any(w.id == sem_id for w in si.on_wait):
                continue
            new_waits = [
                mybir.SyncWait(
                    sync_type=w.sync_type,
                    id=w.id,
                    ant_name=w.ant_name,
                    wait_mode=w.wait_mode,
                    wait_value=rank[w.wait_value],
                    wait_reg=w.wait_reg,
                )
                if w.id == sem_id
                else w
                for w in si.on_wait
            ]
            ins.sync_info = mybir.SyncInfo(
                on_wait=new_waits, on_update=list(si.on_update or [])
            )



def build_program(
    passes=1,
    loop_n=1,
    z_mode="esum",  # "esum" | "mm" | "none"
    skip_exp=False,
    skip_final=False,
    thin_sems=True,
    ps1_bufs=2,
    fw_bufs=3,
    ps2_bufs=2,
    pmz_bf16=True,
):
    nc = bass.Bass()
    xt_d = nc.declare_dram_parameter("xt", [BC, DC, 128, LPAD], BF16, isOutput=False)
    xa_d = nc.declare_dram_parameter("xa", [BC, LC, 128, 513], BF16, isOutput=False)
    ut_d = nc.declare_dram_parameter("ut", [DC, 128, YC], BF16, isOutput=False)
    fw_d = nc.declare_dram_parameter("fw", [JTOT, 128, D], F32, isOutput=False)
    bias_d = nc.declare_dram_parameter("bias", [128, JTOT], F32, isOutput=False)
    out_d = nc.declare_dram_parameter("out", [128, BC * JTOT], F32, isOutput=True)

    Exp = mybir.ActivationFunctionType.Exp
    mult = mybir.AluOpType.mult
    add = mybir.AluOpType.add

    with tile.TileContext(nc) as tc:
        with (
            tc.tile_pool(name="const", bufs=1) as constp,
            tc.tile_pool(name="xpool", bufs=2) as xpool,
            tc.tile_pool(name="etp", bufs=2) as etp,
            tc.tile_pool(name="fwp", bufs=fw_bufs) as fwp,
            tc.tile_pool(name="scr", bufs=2) as scrp,
            tc.tile_pool(name="esump", bufs=2) as esump,
            tc.tile_pool(name="small", bufs=4) as smallp,
            tc.tile_pool(name="ps1", bufs=ps1_bufs, space="PSUM") as ps1,
            tc.tile_pool(name="ps2", bufs=ps2_bufs, space="PSUM") as ps2,
            tc.tile_pool(name="psz", bufs=2, space="PSUM") as psz,
        ):
            ut_t = constp.tile([128, DC, YC], BF16)
            # group-0 slices first so MM1 of the first group can start while
            # the rest of U is still in flight
            for off, width in GROUPS:
                for dc in range(DC):
                    nc.sync.dma_start(
                        ut_t[:, dc, off:off + width], ut_d[dc, :, off:off + width]
                    )
            bias_t = constp.tile([128, JTOT], F32)
            nc.sync.dma_start(bias_t, bias_d[:, :])
            out_t = constp.tile([128, BC * JTOT], F32)
            if skip_final:
                nc.vector.memset(out_t, 0.0)
            ones_t = None
            if z_mode == "esum":
                ones_t = constp.tile([128, 1], BF16 if pmz_bf16 else F32)
                nc.vector.memset(ones_t, 1.0)
            et_const = None
            if skip_exp:
                et_const = constp.tile([128, LC, YG], BF16)
                nc.vector.memset(et_const, 1.0)
            NG = len(GROUPS)

            pm_w = 513 if z_mode == "mm" else 512

            loop_ctx = (
                tc.For_i(0, loop_n, 1) if loop_n > 1 else contextlib.nullcontext()
            )
            with loop_ctx:
             for _pass in range(passes):
              for b in range(BC):
                xt_t = xpool.tile([128, DC, LPAD], BF16, tag="xt")
                for dc in range(DC):
                    nc.sync.dma_start(xt_t[:, dc], xt_d[b, dc])
                xa_t = xpool.tile([128, LC, 513], BF16, tag="xa")
                for lc in range(LC):
                    nc.sync.dma_start(xa_t[:, lc], xa_d[b, lc])

                for g, (goff, gw) in enumerate(GROUPS):
                    et_full = (
                        et_const
                        if skip_exp
                        else etp.tile([128, LC, YG], BF16, tag="et")
                    )
                    et_t = et_full[:, :, :gw]
                    esum_t = None
                    if z_mode == "esum":
                        esum_t = esump.tile([128, YG], F32, tag="esum", name="esum_t")[:, :gw]
                    for lc in range(LC):
                        ps = ps1.tile([128, YG], F32, tag="ps", name="ps")[:, :gw]
                        for dc in range(DC):
                            nc.tensor.matmul(
                                ps,
                                lhsT=xt_t[:, dc, lc * 128:(lc + 1) * 128],
                                rhs=ut_t[:, dc, goff:goff + gw],
                                start=(dc == 0),
                                stop=(dc == DC - 1),
                            )
                        if not skip_exp:
                            nc.scalar.activation(et_t[:, lc], ps, Exp)
                        if z_mode == "esum":
                            if lc == 0:
                                nc.vector.tensor_copy(esum_t, et_t[:, 0])
                            else:
                                nc.vector.tensor_add(esum_t, esum_t, et_t[:, lc])

                    for s in range(gw // 128):
                        j = goff // 128 + s
                        pm = ps2.tile([128, pm_w], F32, tag="pm")
                        for lc in range(LC):
                            lhs = et_t[:, lc, s * 128:(s + 1) * 128]
                            nc.tensor.matmul(
                                pm[:, 0:512],
                                lhsT=lhs,
                                rhs=xa_t[:, lc, 0:512],
                                start=(lc == 0),
                                stop=(lc == LC - 1),
                            )
                            if z_mode == "mm":
                                nc.tensor.matmul(
                                    pm[:, 512:513],
                                    lhsT=lhs,
                                    rhs=xa_t[:, lc, 512:513],
                                    start=(lc == 0),
                                    stop=(lc == LC - 1),
                                    skip_group_check=True,
                                )
                        pmz = None
                        if z_mode == "esum":
                            pmz = psz.tile([128, 1], F32, tag="pmz")
                            if pmz_bf16:
                                esb = scrp.tile(
                                    [128, 128], BF16, tag="esb", name="esb"
                                )
                                nc.vector.tensor_copy(
                                    esb, esum_t[:, s * 128:(s + 1) * 128]
                                )
                                zlhs = esb
                            else:
                                zlhs = esum_t[:, s * 128:(s + 1) * 128]
                            nc.tensor.matmul(
                                pmz,
                                lhsT=zlhs,
                                rhs=ones_t,
                                start=True,
                                stop=True,
                            )
                        if skip_final:
                            continue
                        fw_t = fwp.tile([128, D], F32, tag="fw")
                        nc.sync.dma_start(fw_t, fw_d[j])
                        prod = scrp.tile([128, D], F32, tag="prod")
                        t_acc = smallp.tile([128, 1], F32, tag="t")
                        nc.vector.tensor_tensor(prod, pm[:, 0:512], fw_t, mult)
                        nc.vector.reduce_sum(
                            t_acc, prod, axis=mybir.AxisListType.XYZW
                        )
                        zr = smallp.tile([128, 1], F32, tag="zr")
                        if z_mode == "esum":
                            zs = smallp.tile([128, 1], F32, tag="zs")
                            nc.vector.tensor_scalar_add(zs, pmz, -float(NPAD_L))
                            nc.vector.reciprocal(zr, zs)
                        elif z_mode == "mm":
                            nc.vector.reciprocal(zr, pm[:, 512:513])
                        else:
                            nc.vector.reciprocal(zr, pm[:, 0:1])
                        nc.vector.tensor_scalar(
                            out_t[:, b * JTOT + j: b * JTOT + j + 1],
                            t_acc,
                            zr,
                            bias_t[:, j: j + 1],
                            mult,
                            add,
                        )

            nc.sync.dma_start(out_d[:, :], out_t)

    if thin_sems and loop_n == 1:
        _thin_engine_clock_updates(nc)
    _split_sync_waits(nc)
    return nc


_PROGRAM = None


def _get_program():
    global _PROGRAM
    if _PROGRAM is None:
        _PROGRAM = build_program()
    return _PROGRAM


def make_in_maps(x, U_weight, final_weight, final_bias):
    x = np.asarray(x, dtype=np.float32)
    U_weight = np.asarray(U_weight, dtype=np.float32)
    final_weight = np.asarray(final_weight, dtype=np.float32)
    final_bias = np.asarray(final_bias, dtype=np.float32)

    # x: pad L, cast bf16; natural + transposed + indicator col layouts
    xpad = np.zeros((B, LPAD, D), dtype=nbf16)
    xpad[:, :L] = x.astype(nbf16)
    xa_all = np.zeros((B, LC, 128, 513), dtype=nbf16)
    xa_all[..., :D] = xpad.reshape(B, LC, 128, D)
    ind = np.zeros((LPAD,), dtype=nbf16)
    ind[:L] = 1
    xa_all[..., D] = ind.reshape(LC, 128)[None]
    xt_all = np.ascontiguousarray(
        xpad.transpose(0, 2, 1)
    ).reshape(B, DC, 128, LPAD)

    YF = YSHARDS * YC
    ufl = np.zeros((YF, D), dtype=np.float32)
    ufl[:Y] = U_weight
    fwfl = np.zeros((YF, D), dtype=np.float32)
    fwfl[:Y] = final_weight
    bfl = np.zeros((YF,), dtype=np.float32)
    bfl[:Y] = final_bias

    ut_s, fw_s, bias_s = [], [], []
    for ys in range(YSHARDS):
        u = ufl[ys * YC:(ys + 1) * YC]
        ut_s.append(
            np.ascontiguousarray(u.T.astype(nbf16)).reshape(DC, 128, YC)
        )
        fw_s.append(
            np.ascontiguousarray(fwfl[ys * YC:(ys + 1) * YC]).reshape(JTOT, 128, D)
        )
        bias_s.append(
            np.ascontiguousarray(bfl[ys * YC:(ys + 1) * YC].reshape(JTOT, 128).T)
        )

    in_maps = []
    for c in range(NCORES):
        ys, bs = c // BSHARDS, c % BSHARDS
        in_maps.append(
            {
                "xt": np.ascontiguousarray(xt_all[bs * BC:(bs + 1) * BC]),
                "xa": np.ascontiguousarray(xa_all[bs * BC:(bs + 1) * BC]),
                "ut": ut_s[ys],
                "fw": fw_s[ys],
                "bias": bias_s[ys],
            }
        )
    return in_maps


def gather_output(results):
    yfull = np.zeros((B, YSHARDS * YC), dtype=np.float32)
    for c in range(NCORES):
        ys, bs = c // BSHARDS, c % BSHARDS
        o = np.asarray(results[c]["out"], dtype=np.float32)  # (128, BC*JTOT)
        for b in range(BC):
            blk = o[:, b * JTOT:(b + 1) * JTOT]  # (128, 36)
            yfull[bs * BC + b, ys * YC:(ys + 1) * YC] = blk.T.reshape(-1)
    return yfull[:, :Y]


def run(x, U_weight, final_weight, final_bias, **run_kwargs):
    nc = _get_program()
    in_maps = make_in_maps(x, U_weight, final_weight, final_bias)
    res = run_bass_kernel_spmd(nc, in_maps, core_ids=list(range(NCORES)), **run_kwargs)
    return gather_output(res.results), res


def kernel(x, U_weight, final_weight, final_bias):
    out, _ = run(x, U_weight, final_weight, final_bias)
    return out



# revision 33
# speedup vs baseline: 1.5414x; 1.5414x over previous
"""MultiResCNN output layer (per-label attention) on 8 TRN2 NeuronCores.

Math (reference):
    scores = einsum('yd,bld->byl', U, x)
    alpha  = softmax(scores, axis=l)
    m      = einsum('byl,bld->byd', alpha, x)
    y      = einsum('yd,byd->by', fw, m) + bias

Sharding: 2 label-shards x 4 batch-shards over 8 cores. Each core handles
BC=2 batches x YC=4480 (padded) labels, fully independently (no collectives).

Per-core kernel — fp8 (e4m3) matmuls, perf_mode=DoubleRowSwInterleave (SWI):
256-deep contraction per instruction with a single-pass interleaved weight
load (plain DoubleRow loads the stationary in two 128-column passes and is
weight-path-bound at ~272 ns/instr; SWI + same-stationary adjacency brings
the sustained per-instruction cost down). SWI's hardware weight load reads
the stationary columns REVERSED, absorbed entirely on the host: the
interleaved xt stores l reversed within each 128-chunk (MM1 output rows come
out natural), and ut/ur store y reversed within each 128-block (MM2 output
rows come out natural).

  MM1: scoresT[l, y] = sum_d xT[d, l] * (SU*U)T[d, y]
       lhsT = host-interleaved xt [128, 256] fp8; U pre-scaled by SU=16 on
       host so its fp8 encoding avoids the subnormal range (U ~ ±0.025; e4m3
       min normal is 2^-6).
  exp: eT = exp(scoresT / SU) -- ACT engine, scale=1/SU fused, fp8 output
       written pair-interleaved via a transposed AP ([A B] per y column);
       HW fp8 rounding verified bit-identical to numpy RNE.
  MM2: m[y, d] = sum_l eT[l, y] * x[l, d]; stationary = interleaved eT
       slice [128, 256]. Per l-chunk-pair, the main (xa) and residual (xr)
       matmuls are adjacent and share the stationary, so the PE's weight
       reload is elided/overlapped; xr accumulates into its own PSUM bank.
  Z:   xr's column 511 carries the l-validity indicator, so pmr[:, 511]
       accumulates Z = sum_l e[l, y] for free (the d=511 residual
       compensation is forfeited; ~1/512 of the comp term). Padded l rows
       have indicator 0, so no pad correction is needed.
  fin: y = (sum_d fw*pm + sum_d fwz*pmr) / Z + bias  (DVE; fwz = fw with
       col 511 zeroed so Z is excluded from the dot).

Accuracy ladder (bit-exact numpy sim, verified 3-for-3 against HW):
fp8 alone 1.75e-2; + U prescale 1.55e-2; + unscaled fp8(x - x8) residual
comp in MM2 (comp_x, default) 1.32e-2; + fp8(SU*U - u8) residual comp in
MM1 (comp_u, off by default) 1.01e-2. Gate is 2e-2. All residuals are
host-precomputed and cost only extra fp8 matmuls.

Post-passes on the BIR: _split_sync_waits (this walrus accepts only one sync
wait per instruction) and _thin_engine_clock_updates (drop the PE semaphore
increments no consumer ever waits on; each costs ~26 ns of PE sequencer
time).

Measured (loop-diff, 8 cores): 498 us/pass @ rel err 1.316e-2 (defaults);
649 us @ 1.010e-2 with comp_u="fused"; bf16 baseline was 707-856 us @
9.7e-4.
"""

import contextlib
import os
import sys

import numpy as np

for _p in ("/opt/trn_rl_repo", "/root/.axon_site/_ro/trn_rl_repo"):
    if os.path.isdir(_p) and _p not in sys.path:
        sys.path.insert(0, _p)

import ml_dtypes  # noqa: E402

import concourse.bass as bass  # noqa: E402
import concourse.mybir as mybir  # noqa: E402
import concourse.tile as tile  # noqa: E402
from concourse.bass_utils import run_bass_kernel_spmd  # noqa: E402

BF16 = mybir.dt.bfloat16
F32 = mybir.dt.float32
F8 = mybir.dt.float8e4
DR = mybir.MatmulPerfMode.DoubleRow
SWI = mybir.MatmulPerfMode.DoubleRowSwInterleave
nbf16 = ml_dtypes.bfloat16
nf8 = ml_dtypes.float8_e4m3

B, L, D, Y = 8, 2500, 512, 8921
NCORES = 8
YSHARDS, BSHARDS = 2, 4
BC = B // BSHARDS  # batches per core
YC = 4480  # padded labels per y-shard (2*4480 = 8960 >= 8921)
LPAD = 2560  # padded seq len (20 chunks of 128)
LC = LPAD // 128  # 20 l-chunks
DC = D // 128  # 4 d-chunks
YG = 512  # max y-group (MM1 psum free size)
GROUPS = [(off, min(YG, YC - off)) for off in range(0, YC, YG)]  # 8x512 + 1x384
JTOT = YC // 128  # 35 label subtiles per core
NPAD_L = LPAD - L  # 60: every padded l row contributes exp(0)=1 to eSum
SU = 16.0  # host pre-scale on U; undone in the ACT exp via scale=1/SU


def _split_sync_waits(nc, max_waits=1):
    """This container's walrus rejects >1 sync wait per instruction ("Too many
    sync wait commands"); move excess waits onto chained same-engine NOPs."""
    for fn in nc.m.functions:
        for bb in fn.blocks:
            new_list = []
            for ins in bb.instructions:
                si = ins.sync_info
                if si is not None and si.on_wait is not None and len(si.on_wait) > max_waits:
                    waits = list(si.on_wait)
                    head, tail = waits[:-max_waits], waits[-max_waits:]
                    idx = 0
                    while head:
                        chunk, head = head[:max_waits], head[max_waits:]
                        nop = mybir.InstNoOp(name=f"{ins.name}-ws{idx}", ins=[], outs=[])
                        nop.engine = ins.engine
                        nop.sync_info = mybir.SyncInfo(on_wait=chunk, on_update=[])
                        new_list.append(nop)
                        idx += 1
                    ins.sync_info = mybir.SyncInfo(
                        on_wait=tail,
                        on_update=list(si.on_update) if si.on_update else [],
                    )
                new_list.append(ins)
            bb.instructions[:] = new_list


def _thin_engine_clock_updates(nc, engine_names=("PE",)):
    """Drop per-instruction engine-clock semaphore increments whose tick value
    no wait ever references, renumbering the remaining waits. Each increment
    costs ~26 ns serialized on the engine; matmul-dense programs tick ~6x more
    often than consumers look. Only valid for linear (no control flow)
    programs; callers must not apply it to For_i benchmark builds."""
    all_insts = [
        ins
        for fn in nc.m.functions
        for bb in fn.blocks
        for ins in bb.instructions
    ]
    # semaphores and how they are updated / waited
    upd_engines = {}
    upd_modes = {}
    waited = {}
    for ins in all_insts:
        si = ins.sync_info
        if si is None:
            continue
        for u in si.on_update or []:
            upd_engines.setdefault(u.id, set()).add(str(ins.engine))
            upd_modes.setdefault(u.id, set()).add((u.update_mode, u.update_value))
        for w in si.on_wait or []:
            waited.setdefault(w.id, set())
            if w.wait_mode == "sem-ge-imm":
                waited[w.id].add(w.wait_value)
            else:
                waited[w.id].add("ALL")

    target_engines = {f"EngineType.{e}" for e in engine_names}
    for sem_id, engs in upd_engines.items():
        if len(engs) != 1 or not engs <= target_engines:
            continue
        if upd_modes[sem_id] != {("sem-inc", 1)}:
            continue
        wvals = waited.get(sem_id, set())
        if "ALL" in wvals:
            continue
        # tick numbering in program order
        seq = []
        tick = 0
        for ins in all_insts:
            si = ins.sync_info
            if si and si.on_update:
                for u in si.on_update:
                    if u.id == sem_id:
                        tick += 1
                        seq.append((ins, tick))
        total = tick
        keep = {t for t in wvals if isinstance(t, int)}
        keep.add(total)
        assert all(1 <= t <= total for t in keep), (sem_id, keep, total)
        kept_sorted = sorted(keep)
        rank = {t: i + 1 for i, t in enumerate(kept_sorted)}
        for ins, t in seq:
            if t in keep:
                continue
            si = ins.sync_info
            new_upd = [u for u in si.on_update if u.id != sem_id]
            ins.sync_info = mybir.SyncInfo(
                on_wait=list(si.on_wait or []), on_update=new_upd
            )
        for ins in all_insts:
            si = ins.sync_info
            if not (si and si.on_wait):
                continue
            if not any(w.id == sem_id for w in si.on_wait):
                continue
            new_waits = [
                mybir.SyncWait(
                    sync_type=w.sync_type,
                    id=w.id,
                    ant_name=w.ant_name,
                    wait_mode=w.wait_mode,
                    wait_value=rank[w.wait_value],
                    wait_reg=w.wait_reg,
                )
                if w.id == sem_id
                else w
                for w in si.on_wait
            ]
            ins.sync_info = mybir.SyncInfo(
                on_wait=new_waits, on_update=list(si.on_update or [])
            )


def build_program(
    passes=1,
    loop_n=1,
    comp_x="fused",  # None | "fused"
    comp_u=None,  # None | "fused"
    mm_mode="swi",  # "swi" (DoubleRowSwInterleave; 1-pass weight loads) | "dr"
    skip_exp=False,
    skip_final=False,
    z_mode="mm",  # "mm" (indicator-column PE matmuls) | "none"
    thin_sems=True,
    ps1_bufs=2,
    fw_bufs=3,
    ps2_bufs=2,
    act_merge=2,  # exp over this many l-chunks (PSUM banks) per ACT instr
):
    swi = mm_mode == "swi"
    if swi:
        assert act_merge == 2
    MAIN_PM = SWI if swi else DR
    nc = bass.Bass()
    if swi:
        xt_d = nc.declare_dram_parameter(
            "xt", [BC, DC // 2, 128, LC, 256], F8, isOutput=False
        )
    else:
        xt_d = nc.declare_dram_parameter(
            "xt", [BC, DC, 128, LPAD], F8, isOutput=False
        )
    xa_d = nc.declare_dram_parameter("xa", [BC, LC, 128, 512], F8, isOutput=False)
    ut_d = nc.declare_dram_parameter("ut", [DC, 128, YC], F8, isOutput=False)
    if comp_x:
        xr_d = nc.declare_dram_parameter("xr", [BC, LC, 128, 512], F8, isOutput=False)
        fwz_d = nc.declare_dram_parameter("fwz", [JTOT, 128, D], BF16, isOutput=False)
    if comp_u:
        ur_d = nc.declare_dram_parameter("ur", [DC, 128, YC], F8, isOutput=False)
    if z_mode == "mm" and not comp_x:
        ind_d = nc.declare_dram_parameter("ind", [128, LC, 32], F8, isOutput=False)
    fw_d = nc.declare_dram_parameter("fw", [JTOT, 128, D], BF16, isOutput=False)
    bias_d = nc.declare_dram_parameter("bias", [128, JTOT], F32, isOutput=False)
    out_d = nc.declare_dram_parameter("out", [128, BC * JTOT], F32, isOutput=True)

    Exp = mybir.ActivationFunctionType.Exp
    mult = mybir.AluOpType.mult
    add = mybir.AluOpType.add
    AM = act_merge

    with tile.TileContext(nc) as tc:
        with contextlib.ExitStack() as stack:
            ent = stack.enter_context
            constp = ent(tc.tile_pool(name="const", bufs=1))
            xpool = ent(tc.tile_pool(name="xpool", bufs=2))
            etp = ent(tc.tile_pool(name="etp", bufs=2))
            fwp = ent(tc.tile_pool(name="fwp", bufs=2 * fw_bufs if comp_x else fw_bufs))
            scrp = ent(tc.tile_pool(name="scr", bufs=3))
            smallp = ent(tc.tile_pool(name="small", bufs=6))
            ps1 = ent(tc.tile_pool(name="ps1", bufs=ps1_bufs, space="PSUM"))
            ps2 = ent(tc.tile_pool(name="ps2", bufs=ps2_bufs, space="PSUM"))
            psz = psr = None
            if comp_x:
                psr = ent(tc.tile_pool(name="psr", bufs=2, space="PSUM"))
            else:
                psz = ent(tc.tile_pool(name="psz", bufs=1, space="PSUM"))
            ut_t = constp.tile([128, DC, YC], F8)
            ur_t = (
                constp.tile([128, DC, YC], F8, name="ur_t") if comp_u else None
            )
            # group-0 slices first so MM1 of the first group can start while
            # the rest of U is still in flight
            for off, width in GROUPS:
                for dc in range(DC):
                    nc.sync.dma_start(
                        ut_t[:, dc, off:off + width], ut_d[dc, :, off:off + width]
                    )
                    if comp_u:
                        nc.sync.dma_start(
                            ur_t[:, dc, off:off + width],
                            ur_d[dc, :, off:off + width],
                        )
            bias_t = constp.tile([128, JTOT], F32)
            nc.sync.dma_start(bias_t, bias_d[:, :])
            ind_t = ident1 = None
            if z_mode == "mm" and not comp_x:
                ind_t = constp.tile([128, LC, 32], F8, name="ind_t")
                nc.sync.dma_start(ind_t, ind_d[:])
                ident1 = constp.tile([1, 1], F32, name="ident1")
                nc.vector.memset(ident1, 1.0)
            out_t = constp.tile([128, BC * JTOT], F32)
            if skip_final:
                nc.vector.memset(out_t, 0.0)
            et_const = None
            if skip_exp:
                if swi:
                    et_const = constp.tile([128, LC // 2, YG, 2], F8)
                else:
                    et_const = constp.tile([128, LC, YG], F8)
                nc.vector.memset(et_const, 1.0)

            loop_ctx = (
                tc.For_i(0, loop_n, 1) if loop_n > 1 else contextlib.nullcontext()
            )
            with loop_ctx:
             for _pass in range(passes):
              for b in range(BC):
                if swi:
                    xt_t = xpool.tile([128, DC // 2, LC, 256], F8, tag="xt")
                    for dp in range(DC // 2):
                        nc.sync.dma_start(xt_t[:, dp], xt_d[b, dp])
                else:
                    xt_t = xpool.tile([128, DC, LPAD], F8, tag="xt")
                    for dc in range(DC):
                        nc.sync.dma_start(xt_t[:, dc], xt_d[b, dc])
                xa_t = xpool.tile([128, LC, 512], F8, tag="xa")
                for lc in range(LC):
                    nc.sync.dma_start(xa_t[:, lc], xa_d[b, lc])
                if comp_x:
                    xr_t = xpool.tile([128, LC, 512], F8, tag="xr")
                    for lc in range(LC):
                        nc.sync.dma_start(xr_t[:, lc], xr_d[b, lc])

                for g, (goff, gw) in enumerate(GROUPS):
                    if skip_exp:
                        et_full = et_const
                    elif swi:
                        et_full = etp.tile([128, LC // 2, YG, 2], F8, tag="et")
                    else:
                        et_full = etp.tile([128, LC, YG], F8, tag="et")
                    if swi:
                        et_t = et_full[:, :, :gw, :]
                    else:
                        et_t = et_full[:, :, :gw]
                    gsl = slice(goff, goff + gw)
                    for lcm in range(LC // AM):
                        ps_full = ps1.tile(
                            [128, AM, YG], F32, tag="ps", name="ps"
                        )
                        for a in range(AM):
                            lc = lcm * AM + a
                            ps = ps_full[:, a, :gw]
                            lsl = slice(lc * 128, (lc + 1) * 128)
                            # main and comp_u instructions adjacent per dp:
                            # they share the same stationary (xt slice), so
                            # the PE's weight reload is elided/overlapped
                            for dp in range(DC // 2):
                                lhs1 = (
                                    xt_t[:, dp, lc]
                                    if swi
                                    else xt_t[:, 2 * dp:2 * dp + 2, lsl]
                                )
                                last_dp = dp == DC // 2 - 1
                                nc.tensor.matmul(
                                    ps,
                                    lhsT=lhs1,
                                    rhs=ut_t[:, 2 * dp:2 * dp + 2, gsl],
                                    start=(dp == 0),
                                    stop=(last_dp and not comp_u),
                                    perf_mode=MAIN_PM,
                                    skip_group_check=True,
                                )
                                if comp_u:
                                    nc.tensor.matmul(
                                        ps,
                                        lhsT=lhs1,
                                        rhs=ur_t[:, 2 * dp:2 * dp + 2, gsl],
                                        start=False,
                                        stop=last_dp,
                                        perf_mode=MAIN_PM,
                                        skip_group_check=True,
                                    )
                        if not skip_exp:
                            act_out = (
                                et_t[:, lcm, :, :].transpose([0, 2, 1])
                                if swi
                                else et_t[:, lcm * AM:(lcm + 1) * AM]
                            )
                            nc.scalar.activation(
                                act_out,
                                ps_full[:, :, :gw],
                                Exp,
                                scale=1.0 / SU,
                            )

                    zrow = None
                    if z_mode == "mm" and not comp_x:
                        # Z row for the whole group: ones-stationary DR
                        # matmuls (2-column weight loads) accumulating
                        # Z[., y] over l-chunk pairs, then one DVE copy to
                        # SBUF; per-subtile PE transpose puts Z on partitions.
                        pz = psz.tile([32, YG], F32, tag="pz", name="pz")[:, :gw]
                        for lp in range(LC // 2):
                            lpsl = slice(2 * lp, 2 * lp + 2)
                            zrhs = (
                                et_t[:, lp, :, :].transpose([0, 2, 1])
                                if swi
                                else et_t[:, lpsl]
                            )
                            nc.tensor.matmul(
                                pz,
                                lhsT=ind_t[:, lpsl, 0:32],
                                rhs=zrhs,
                                start=(lp == 0),
                                stop=(lp == LC // 2 - 1),
                                perf_mode=DR,
                                skip_group_check=True,
                            )
                        zrow = scrp.tile(
                            [1, YG], F32, tag="zrow", name="zrow"
                        )[:, :gw]
                        if swi:
                            # pz columns carry the reversed-ut y order;
                            # un-reverse per 128-subtile during the copy
                            nsub = gw // 128
                            zsrc = pz[0:1].rearrange(
                                "p (s q) -> p s q", q=128
                            )[:, :, ::-1]
                            zdst = zrow.rearrange("p (s q) -> p s q", q=128)
                            nc.vector.tensor_copy(zdst, zsrc)
                        else:
                            nc.vector.tensor_copy(zrow, pz[0:1])

                    for s in range(gw // 128):
                        j = goff // 128 + s
                        ssl = slice(s * 128, (s + 1) * 128)
                        pm = ps2.tile([128, 512], F32, tag="pm")
                        pmr = (
                            psr.tile([128, 512], F32, tag="pmr", name="pmr")
                            if comp_x
                            else None
                        )
                        for lp in range(LC // 2):
                            lpsl = slice(2 * lp, 2 * lp + 2)
                            lhs2 = (
                                et_t[:, lp, ssl, :].rearrange("p y a -> p (y a)")
                                if swi
                                else et_t[:, lpsl, ssl]
                            )
                            last_lp = lp == LC // 2 - 1
                            nc.tensor.matmul(
                                pm,
                                lhsT=lhs2,
                                rhs=xa_t[:, lpsl],
                                start=(lp == 0),
                                stop=last_lp,
                                perf_mode=MAIN_PM,
                                skip_group_check=True,
                            )
                            if comp_x:
                                # same stationary as the main matmul; xr's
                                # col 511 carries the l-validity indicator,
                                # so pmr[:, 511] accumulates Z for free
                                nc.tensor.matmul(
                                    pmr,
                                    lhsT=lhs2,
                                    rhs=xr_t[:, lpsl],
                                    start=(lp == 0),
                                    stop=last_lp,
                                    perf_mode=MAIN_PM,
                                    skip_group_check=True,
                                )
                        if skip_final:
                            continue
                        fw_t = fwp.tile([128, D], BF16, tag="fw")
                        nc.sync.dma_start(fw_t, fw_d[j])
                        prod = scrp.tile([128, D], F32, tag="prod")
                        t_acc = smallp.tile([128, 1], F32, tag="t")
                        nc.vector.tensor_tensor(prod, pm, fw_t, mult)
                        nc.vector.reduce_sum(
                            t_acc, prod, axis=mybir.AxisListType.XYZW
                        )
                        zr = smallp.tile([128, 1], F32, tag="zr")
                        if comp_x:
                            fwz_t = fwp.tile([128, D], BF16, tag="fwz")
                            nc.sync.dma_start(fwz_t, fwz_d[j])
                            prod2 = scrp.tile([128, D], F32, tag="prod2")
                            t2 = smallp.tile([128, 1], F32, tag="t2")
                            nc.vector.tensor_tensor(prod2, pmr, fwz_t, mult)
                            nc.vector.reduce_sum(
                                t2, prod2, axis=mybir.AxisListType.XYZW
                            )
                            t_fin = smallp.tile([128, 1], F32, tag="tf")
                            nc.vector.tensor_add(t_fin, t_acc, t2)
                            nc.vector.reciprocal(zr, pmr[:, 511:512])
                        elif z_mode == "mm":
                            t_fin = t_acc
                            zT = psz.tile([128, 4], F32, tag="zT", name="zT")
                            nc.tensor.transpose(
                                zT[:, 0:1], zrow[:, ssl], ident1
                            )
                            nc.vector.reciprocal(zr, zT[:, 0:1])
                        else:
                            t_fin = t_acc
                            nc.vector.reciprocal(zr, pm[:, 0:1])
                        nc.vector.tensor_scalar(
                            out_t[:, b * JTOT + j: b * JTOT + j + 1],
                            t_fin,
                            zr,
                            bias_t[:, j: j + 1],
                            mult,
                            add,
                        )

             nc.sync.dma_start(out_d[:, :], out_t)

    if thin_sems and loop_n == 1:
        _thin_engine_clock_updates(nc)
    _split_sync_waits(nc)
    return nc


_PROGRAM = None


def _get_program():
    global _PROGRAM
    if _PROGRAM is None:
        _PROGRAM = build_program()
    return _PROGRAM


def make_in_maps(x, U_weight, final_weight, final_bias,
                 comp_x="fused", comp_u=None, z_mode="mm", mm_mode="swi"):
    swi = mm_mode == "swi"
    x = np.asarray(x, dtype=np.float32)
    U_weight = np.asarray(U_weight, dtype=np.float32)
    final_weight = np.asarray(final_weight, dtype=np.float32)
    final_bias = np.asarray(final_bias, dtype=np.float32)

    # x: pad L, quantize fp8; natural + transposed layouts (+ residual).
    # SWI mode: MM1's software-interleaved stationary is loaded with its
    # columns (l within a 128-chunk) reversed by the PE, so every l-indexed
    # tensor (xa/xr/ind and the interleaved xt itself) stores l reversed
    # within each chunk; likewise MM2's stationary (eT) gets its columns
    # (y within a 128-subtile) reversed, absorbed by reversing ut/ur
    # y-blocks and un-reversing fw/bias/output on the host.
    xpad = np.zeros((B, LPAD, D), dtype=np.float32)
    xpad[:, :L] = x
    x8 = xpad.astype(nf8)
    x8f = x8.astype(np.float32)
    xrsd = (xpad - x8f).astype(nf8)
    xa_all = np.ascontiguousarray(x8.reshape(B, LC, 128, D))
    if swi:
        # SWI loads reverse the stationary's columns, so the interleaved
        # xt stores l reversed within each 128-chunk; MM1's output rows
        # then come out in natural l order and xa/xr/ind stay natural.
        # xt_il[b, dp, d_p, lc, 2*lr + a]
        #   = x8[b, lc*128 + (127-lr), dp*256 + a*128 + d_p]
        x8rev = np.ascontiguousarray(
            x8.reshape(B, LC, 128, D)[:, :, ::-1, :]
        ).reshape(B, LPAD, D)
        t = x8rev.reshape(B, LC, 128, DC // 2, 2, 128)
        xt_all = np.ascontiguousarray(
            t.transpose(0, 3, 5, 1, 2, 4)
        ).reshape(B, DC // 2, 128, LC, 256)
    else:
        xt_all = np.ascontiguousarray(
            x8.transpose(0, 2, 1)
        ).reshape(B, DC, 128, LPAD)
    xr_all = None
    if comp_x:
        xr_all = xrsd.reshape(B, LC, 128, D).copy()
        # col 511 carries the l-validity indicator so pmr[:, 511] = Z;
        # that column's x-residual compensation is forfeited (1/512 of it)
        lval = (np.arange(LPAD).reshape(LC, 128) < L).astype(nf8)
        xr_all[:, :, :, 511] = lval[None]
        xr_all = np.ascontiguousarray(xr_all)

    YF = YSHARDS * YC
    ufl = np.zeros((YF, D), dtype=np.float32)
    ufl[:Y] = U_weight * SU
    u8 = ufl.astype(nf8)
    u8f = u8.astype(np.float32)
    ursd = (ufl - u8f).astype(nf8)

    def yrev(a):
        # reverse y within each 128-block (rows of a [YC, ...] array):
        # MM2's SWI stationary load reverses eT's columns per subtile, so
        # storing ut/ur with y-blocks pre-reversed makes pm rows natural.
        return np.ascontiguousarray(
            a.reshape(JTOT, 128, *a.shape[1:])[:, ::-1]
        ).reshape(a.shape)

    fwfl = np.zeros((YF, D), dtype=np.float32)
    fwfl[:Y] = final_weight
    bfl = np.zeros((YF,), dtype=np.float32)
    bfl[:Y] = final_bias

    ut_s, ur_s, fw_s, fwz_s, bias_s = [], [], [], [], []
    for ys in range(YSHARDS):
        sl = slice(ys * YC, (ys + 1) * YC)
        u_sh, ur_sh = u8[sl], ursd[sl]
        fw_sh, b_sh = fwfl[sl], bfl[sl]
        if swi:
            u_sh, ur_sh = yrev(u_sh), yrev(ur_sh)
        ut_s.append(np.ascontiguousarray(u_sh.T).reshape(DC, 128, YC))
        if comp_u:
            ur_s.append(np.ascontiguousarray(ur_sh.T).reshape(DC, 128, YC))
        fw_s.append(
            np.ascontiguousarray(fw_sh.astype(nbf16)).reshape(JTOT, 128, D)
        )
        if comp_x:
            fwz = fw_sh.astype(nbf16).copy()
            fwz[:, 511] = 0
            fwz_s.append(np.ascontiguousarray(fwz).reshape(JTOT, 128, D))
        bias_s.append(np.ascontiguousarray(b_sh.reshape(JTOT, 128).T))

    ind = np.zeros((128, LC, 32), dtype=nf8)
    lidx = np.arange(LPAD).reshape(LC, 128)  # [lc, p] -> l
    ind[:, :, 0] = (lidx.T < L).astype(nf8)

    in_maps = []
    for c in range(NCORES):
        ys, bs = c // BSHARDS, c % BSHARDS
        m = {
            "xt": np.ascontiguousarray(xt_all[bs * BC:(bs + 1) * BC]),
            "xa": np.ascontiguousarray(xa_all[bs * BC:(bs + 1) * BC]),
            "ut": ut_s[ys],
            "fw": fw_s[ys],
            "bias": bias_s[ys],
        }
        if comp_x:
            m["xr"] = np.ascontiguousarray(xr_all[bs * BC:(bs + 1) * BC])
            m["fwz"] = fwz_s[ys]
        if comp_u:
            m["ur"] = ur_s[ys]
        if z_mode == "mm" and not comp_x:
            m["ind"] = ind.copy()
        in_maps.append(m)
    return in_maps


def gather_output(results, mm_mode="swi"):
    yfull = np.zeros((B, YSHARDS * YC), dtype=np.float32)
    for c in range(NCORES):
        ys, bs = c // BSHARDS, c % BSHARDS
        o = np.asarray(results[c]["out"], dtype=np.float32)  # (128, BC*JTOT)
        for b in range(BC):
            blk = o[:, b * JTOT:(b + 1) * JTOT]  # (128, 35)
            yfull[bs * BC + b, ys * YC:(ys + 1) * YC] = blk.T.reshape(-1)
    return yfull[:, :Y]


def run(x, U_weight, final_weight, final_bias, **run_kwargs):
    nc = _get_program()
    in_maps = make_in_maps(x, U_weight, final_weight, final_bias)
    res = run_bass_kernel_spmd(nc, in_maps, core_ids=list(range(NCORES)), **run_kwargs)
    return gather_output(res.results), res


def kernel(x, U_weight, final_weight, final_bias):
    out, _ = run(x, U_weight, final_weight, final_bias)
    return out


# revision 38
# speedup vs baseline: 3.1379x; 2.0358x over previous
"""MultiResCNN output layer (per-label attention) on 8 TRN2 NeuronCores.

Math (reference):
    scores = einsum('yd,bld->byl', U, x)
    alpha  = softmax(scores, axis=l)
    m      = einsum('byl,bld->byd', alpha, x)
    y      = einsum('yd,byd->by', fw, m) + bias

Sharding: 2 label-shards x 4 batch-shards over 8 cores. Each core handles
BC=2 batches x YC=4480 (padded) labels, fully independently (no collectives).

Per-core kernel — fp8 (e4m3) matmuls, perf_mode=DoubleRowSwInterleave (SWI):
256-deep contraction per instruction with a single-pass interleaved weight
load (plain DoubleRow loads the stationary in two 128-column passes and is
weight-path-bound at ~272 ns/instr; SWI + same-stationary adjacency brings
the sustained per-instruction cost down). SWI's hardware weight load reads
the stationary columns REVERSED, absorbed entirely on the host: the
interleaved xt stores l reversed within each 128-chunk (MM1 output rows come
out natural), and ut/ur store y reversed within each 128-block (MM2 output
rows come out natural).

  MM1: scoresT[l, y] = sum_d xT[d, l] * (SU*U)T[d, y]
       lhsT = host-interleaved xt [128, 256] fp8; U pre-scaled by SU=16 on
       host so its fp8 encoding avoids the subnormal range (U ~ ±0.025; e4m3
       min normal is 2^-6).
  exp: eT = exp(scoresT / SU) -- ACT engine, scale=1/SU fused, fp8 output
       written pair-interleaved via a transposed AP ([A B] per y column);
       HW fp8 rounding verified bit-identical to numpy RNE.
  MM2: m[y, d] = sum_l eT[l, y] * x[l, d]; stationary = interleaved eT
       slice [128, 256]. Per l-chunk-pair, the main (xa) and residual (xr)
       matmuls are adjacent and share the stationary, so the PE's weight
       reload is elided/overlapped; xr accumulates into its own PSUM bank.
  Z:   xr's column 511 carries the l-validity indicator, so pmr[:, 511]
       accumulates Z = sum_l e[l, y] for free (the d=511 residual
       compensation is forfeited; ~1/512 of the comp term). Padded l rows
       have indicator 0, so no pad correction is needed.
  fin: y = (sum_d fw*pm + sum_d fwz*pmr) / Z + bias  (DVE; fwz = fw with
       col 511 zeroed so Z is excluded from the dot).

Accuracy ladder (bit-exact numpy sim, verified 3-for-3 against HW):
fp8 alone 1.75e-2; + U prescale 1.55e-2; + unscaled fp8(x - x8) residual
comp in MM2 (comp_x, default) 1.32e-2; + fp8(SU*U - u8) residual comp in
MM1 (comp_u, off by default) 1.01e-2. Gate is 2e-2. All residuals are
host-precomputed and cost only extra fp8 matmuls.

Post-passes on the BIR: _split_sync_waits (this walrus accepts only one sync
wait per instruction) and _thin_engine_clock_updates (drop the PE semaphore
increments no consumer ever waits on; each costs ~26 ns of PE sequencer
time).

MM1 processes y-groups in chunks of 2 with the l-loop inner, so
consecutive MM1 instructions also share their stationary across the two
groups' moving operands; fw/fwz are preloaded to SBUF once instead of
per-subtile DMAs.

Measured (loop-diff, 8 cores): 375 us/pass @ rel err 1.316e-2 (defaults;
498 us before the MM1 group-chunking); bf16 baseline was 707-856 us @
9.7e-4. comp_u="fused" option reaches 1.01e-2 at extra MM1 cost.
"""

import contextlib
import os
import sys

import numpy as np

for _p in ("/opt/trn_rl_repo", "/root/.axon_site/_ro/trn_rl_repo"):
    if os.path.isdir(_p) and _p not in sys.path:
        sys.path.insert(0, _p)

import ml_dtypes  # noqa: E402

import concourse.bass as bass  # noqa: E402
import concourse.mybir as mybir  # noqa: E402
import concourse.tile as tile  # noqa: E402
from concourse.bass_utils import run_bass_kernel_spmd  # noqa: E402

BF16 = mybir.dt.bfloat16
F32 = mybir.dt.float32
F8 = mybir.dt.float8e4
DR = mybir.MatmulPerfMode.DoubleRow
SWI = mybir.MatmulPerfMode.DoubleRowSwInterleave
nbf16 = ml_dtypes.bfloat16
nf8 = ml_dtypes.float8_e4m3

B, L, D, Y = 8, 2500, 512, 8921
NCORES = 8
YSHARDS, BSHARDS = 2, 4
BC = B // BSHARDS  # batches per core
YC = 4480  # padded labels per y-shard (2*4480 = 8960 >= 8921)
LPAD = 2560  # padded seq len (20 chunks of 128)
LC = LPAD // 128  # 20 l-chunks
DC = D // 128  # 4 d-chunks
YG = 512  # max y-group (MM1 psum free size)
GROUPS = [(off, min(YG, YC - off)) for off in range(0, YC, YG)]  # 8x512 + 1x384
JTOT = YC // 128  # 35 label subtiles per core
NPAD_L = LPAD - L  # 60: every padded l row contributes exp(0)=1 to eSum
SU = 16.0  # host pre-scale on U; undone in the ACT exp via scale=1/SU


def _split_sync_waits(nc, max_waits=1):
    """This container's walrus rejects >1 sync wait per instruction ("Too many
    sync wait commands"); move excess waits onto chained same-engine NOPs."""
    for fn in nc.m.functions:
        for bb in fn.blocks:
            new_list = []
            for ins in bb.instructions:
                si = ins.sync_info
                if si is not None and si.on_wait is not None and len(si.on_wait) > max_waits:
                    waits = list(si.on_wait)
                    head, tail = waits[:-max_waits], waits[-max_waits:]
                    idx = 0
                    while head:
                        chunk, head = head[:max_waits], head[max_waits:]
                        nop = mybir.InstNoOp(name=f"{ins.name}-ws{idx}", ins=[], outs=[])
                        nop.engine = ins.engine
                        nop.sync_info = mybir.SyncInfo(on_wait=chunk, on_update=[])
                        new_list.append(nop)
                        idx += 1
                    ins.sync_info = mybir.SyncInfo(
                        on_wait=tail,
                        on_update=list(si.on_update) if si.on_update else [],
                    )
                new_list.append(ins)
            bb.instructions[:] = new_list


def _thin_engine_clock_updates(nc, engine_names=("PE",)):
    """Drop per-instruction engine-clock semaphore increments whose tick value
    no wait ever references, renumbering the remaining waits. Each increment
    costs ~26 ns serialized on the engine; matmul-dense programs tick ~6x more
    often than consumers look. Only valid for linear (no control flow)
    programs; callers must not apply it to For_i benchmark builds."""
    all_insts = [
        ins
        for fn in nc.m.functions
        for bb in fn.blocks
        for ins in bb.instructions
    ]
    # semaphores and how they are updated / waited
    upd_engines = {}
    upd_modes = {}
    waited = {}
    for ins in all_insts:
        si = ins.sync_info
        if si is None:
            continue
        for u in si.on_update or []:
            upd_engines.setdefault(u.id, set()).add(str(ins.engine))
            upd_modes.setdefault(u.id, set()).add((u.update_mode, u.update_value))
        for w in si.on_wait or []:
            waited.setdefault(w.id, set())
            if w.wait_mode == "sem-ge-imm":
                waited[w.id].add(w.wait_value)
            else:
                waited[w.id].add("ALL")

    target_engines = {f"EngineType.{e}" for e in engine_names}
    for sem_id, engs in upd_engines.items():
        if len(engs) != 1 or not engs <= target_engines:
            continue
        if upd_modes[sem_id] != {("sem-inc", 1)}:
            continue
        wvals = waited.get(sem_id, set())
        if "ALL" in wvals:
            continue
        # tick numbering in program order
        seq = []
        tick = 0
        for ins in all_insts:
            si = ins.sync_info
            if si and si.on_update:
                for u in si.on_update:
                    if u.id == sem_id:
                        tick += 1
                        seq.append((ins, tick))
        total = tick
        keep = {t for t in wvals if isinstance(t, int)}
        keep.add(total)
        assert all(1 <= t <= total for t in keep), (sem_id, keep, total)
        kept_sorted = sorted(keep)
        rank = {t: i + 1 for i, t in enumerate(kept_sorted)}
        for ins, t in seq:
            if t in keep:
                continue
            si = ins.sync_info
            new_upd = [u for u in si.on_update if u.id != sem_id]
            ins.sync_info = mybir.SyncInfo(
                on_wait=list(si.on_wait or []), on_update=new_upd
            )
        for ins in all_insts:
            si = ins.sync_info
            if not (si and si.on_wait):
                continue
            if not any(w.id == sem_id for w in si.on_wait):
                continue
            new_waits = [
                mybir.SyncWait(
                    sync_type=w.sync_type,
                    id=w.id,
                    ant_name=w.ant_name,
                    wait_mode=w.wait_mode,
                    wait_value=rank[w.wait_value],
                    wait_reg=w.wait_reg,
                )
                if w.id == sem_id
                else w
                for w in si.on_wait
            ]
            ins.sync_info = mybir.SyncInfo(
                on_wait=new_waits, on_update=list(si.on_update or [])
            )


def build_program(
    passes=1,
    loop_n=1,
    comp_x="fused",  # None | "fused"
    comp_u=None,  # None | "fused"
    mm_mode="swi",  # "swi" (DoubleRowSwInterleave; 1-pass weight loads) | "dr"
    skip_exp=False,
    skip_final=False,
    z_mode="mm",  # "mm" (indicator-column PE matmuls) | "none"
    thin_sems=True,
    ps1_bufs=2,
    fw_bufs=3,
    ps2_bufs=2,
    act_merge=2,  # exp over this many l-chunks (PSUM banks) per ACT instr
):
    swi = mm_mode == "swi"
    if swi:
        assert act_merge == 2
    MAIN_PM = SWI if swi else DR
    nc = bass.Bass()
    if swi:
        xt_d = nc.declare_dram_parameter(
            "xt", [BC, DC // 2, 128, LC, 256], F8, isOutput=False
        )
    else:
        xt_d = nc.declare_dram_parameter(
            "xt", [BC, DC, 128, LPAD], F8, isOutput=False
        )
    xa_d = nc.declare_dram_parameter("xa", [BC, LC, 128, 512], F8, isOutput=False)
    ut_d = nc.declare_dram_parameter("ut", [DC, 128, YC], F8, isOutput=False)
    if comp_x:
        xr_d = nc.declare_dram_parameter("xr", [BC, LC, 128, 512], F8, isOutput=False)
        fwz_d = nc.declare_dram_parameter("fwz", [JTOT, 128, D], BF16, isOutput=False)
    if comp_u:
        ur_d = nc.declare_dram_parameter("ur", [DC, 128, YC], F8, isOutput=False)
    if z_mode == "mm" and not comp_x:
        ind_d = nc.declare_dram_parameter("ind", [128, LC, 32], F8, isOutput=False)
    fw_d = nc.declare_dram_parameter("fw", [JTOT, 128, D], BF16, isOutput=False)
    bias_d = nc.declare_dram_parameter("bias", [128, JTOT], F32, isOutput=False)
    out_d = nc.declare_dram_parameter("out", [128, BC * JTOT], F32, isOutput=True)

    Exp = mybir.ActivationFunctionType.Exp
    mult = mybir.AluOpType.mult
    add = mybir.AluOpType.add
    AM = act_merge

    with tile.TileContext(nc) as tc:
        with contextlib.ExitStack() as stack:
            ent = stack.enter_context
            constp = ent(tc.tile_pool(name="const", bufs=1))
            xpool = ent(tc.tile_pool(name="xpool", bufs=2))
            etp = ent(tc.tile_pool(name="etp", bufs=2))
            fwp = ent(tc.tile_pool(name="fwp", bufs=2 * fw_bufs if comp_x else fw_bufs))
            scrp = ent(tc.tile_pool(name="scr", bufs=3))
            smallp = ent(tc.tile_pool(name="small", bufs=6))
            ps1 = ent(tc.tile_pool(name="ps1", bufs=ps1_bufs, space="PSUM"))
            ps2 = ent(tc.tile_pool(name="ps2", bufs=ps2_bufs, space="PSUM"))
            psz = psr = None
            if comp_x:
                psr = ent(tc.tile_pool(name="psr", bufs=2, space="PSUM"))
            else:
                psz = ent(tc.tile_pool(name="psz", bufs=1, space="PSUM"))
            ut_t = constp.tile([128, DC, YC], F8)
            ur_t = (
                constp.tile([128, DC, YC], F8, name="ur_t") if comp_u else None
            )
            # preload the full final-weight tensors once (replaces per-subtile
            # DMAs inside the main loop)
            fw_all = constp.tile([128, JTOT, D], BF16, name="fw_all")
            for j in range(JTOT):
                nc.sync.dma_start(fw_all[:, j], fw_d[j])
            fwz_all = None
            if comp_x:
                fwz_all = constp.tile([128, JTOT, D], BF16, name="fwz_all")
                for j in range(JTOT):
                    nc.sync.dma_start(fwz_all[:, j], fwz_d[j])
            # group-0 slices first so MM1 of the first group can start while
            # the rest of U is still in flight
            for off, width in GROUPS:
                for dc in range(DC):
                    nc.sync.dma_start(
                        ut_t[:, dc, off:off + width], ut_d[dc, :, off:off + width]
                    )
                    if comp_u:
                        nc.sync.dma_start(
                            ur_t[:, dc, off:off + width],
                            ur_d[dc, :, off:off + width],
                        )
            bias_t = constp.tile([128, JTOT], F32)
            nc.sync.dma_start(bias_t, bias_d[:, :])
            ind_t = ident1 = None
            if z_mode == "mm" and not comp_x:
                ind_t = constp.tile([128, LC, 32], F8, name="ind_t")
                nc.sync.dma_start(ind_t, ind_d[:])
                ident1 = constp.tile([1, 1], F32, name="ident1")
                nc.vector.memset(ident1, 1.0)
            out_t = constp.tile([128, BC * JTOT], F32)
            if skip_final:
                nc.vector.memset(out_t, 0.0)
            et_const = None
            if skip_exp:
                if swi:
                    et_const = constp.tile([128, LC // 2, YG, 2], F8)
                else:
                    et_const = constp.tile([128, LC, YG], F8)
                nc.vector.memset(et_const, 1.0)

            loop_ctx = (
                tc.For_i(0, loop_n, 1) if loop_n > 1 else contextlib.nullcontext()
            )
            with loop_ctx:
             for _pass in range(passes):
              for b in range(BC):
                if swi:
                    xt_t = xpool.tile([128, DC // 2, LC, 256], F8, tag="xt")
                    for dp in range(DC // 2):
                        nc.sync.dma_start(xt_t[:, dp], xt_d[b, dp])
                else:
                    xt_t = xpool.tile([128, DC, LPAD], F8, tag="xt")
                    for dc in range(DC):
                        nc.sync.dma_start(xt_t[:, dc], xt_d[b, dc])
                xa_t = xpool.tile([128, LC, 512], F8, tag="xa")
                for lc in range(LC):
                    nc.sync.dma_start(xa_t[:, lc], xa_d[b, lc])
                if comp_x:
                    xr_t = xpool.tile([128, LC, 512], F8, tag="xr")
                    for lc in range(LC):
                        nc.sync.dma_start(xr_t[:, lc], xr_d[b, lc])

                CH = 2 if (swi and comp_x) else 1
                for c0 in range(0, len(GROUPS), CH):
                    chunk = GROUPS[c0:c0 + CH]
                    ets = []
                    for gi, (goff, gw) in enumerate(chunk):
                        if skip_exp:
                            ef = et_const
                        elif swi:
                            ef = etp.tile(
                                [128, LC // 2, YG, 2], F8, tag=f"et{gi}",
                                name="ef",
                            )
                        else:
                            ef = etp.tile(
                                [128, LC, YG], F8, tag=f"et{gi}", name="ef"
                            )
                        ets.append(ef[:, :, :gw, :] if swi else ef[:, :, :gw])
                    # MM1: groups of the chunk interleaved so consecutive
                    # instructions share the same stationary (xt slice) and
                    # the PE's weight reload is elided/overlapped; comp_u
                    # instructions are adjacent for the same reason.
                    for lc in range(LC):
                        pss = []
                        for gi, (goff, gw) in enumerate(chunk):
                            pss.append(
                                ps1.tile(
                                    [128, YG], F32, tag=f"ps{gi}", name="psg"
                                )[:, :gw]
                            )
                        lsl = slice(lc * 128, (lc + 1) * 128)
                        for dp in range(DC // 2):
                            lhs1 = (
                                xt_t[:, dp, lc]
                                if swi
                                else xt_t[:, 2 * dp:2 * dp + 2, lsl]
                            )
                            last_dp = dp == DC // 2 - 1
                            for gi, (goff, gw) in enumerate(chunk):
                                gsl = slice(goff, goff + gw)
                                nc.tensor.matmul(
                                    pss[gi],
                                    lhsT=lhs1,
                                    rhs=ut_t[:, 2 * dp:2 * dp + 2, gsl],
                                    start=(dp == 0),
                                    stop=(last_dp and not comp_u),
                                    perf_mode=MAIN_PM,
                                    skip_group_check=True,
                                )
                                if comp_u:
                                    nc.tensor.matmul(
                                        pss[gi],
                                        lhsT=lhs1,
                                        rhs=ur_t[:, 2 * dp:2 * dp + 2, gsl],
                                        start=False,
                                        stop=last_dp,
                                        perf_mode=MAIN_PM,
                                        skip_group_check=True,
                                    )
                        if not skip_exp:
                            for gi in range(len(chunk)):
                                act_out = (
                                    ets[gi][:, lc // 2, :, lc % 2]
                                    if swi
                                    else ets[gi][:, lc]
                                )
                                nc.scalar.activation(
                                    act_out, pss[gi], Exp, scale=1.0 / SU
                                )

                    for gi, (goff, gw) in enumerate(chunk):
                      et_t = ets[gi]
                      zrow = None
                      if z_mode == "mm" and not comp_x:
                        # Z row for the whole group: ones-stationary DR
                        # matmuls (2-column weight loads) accumulating
                        # Z[., y] over l-chunk pairs, then one DVE copy to
                        # SBUF; per-subtile PE transpose puts Z on partitions.
                        pz = psz.tile([32, YG], F32, tag="pz", name="pz")[:, :gw]
                        for lp in range(LC // 2):
                            lpsl = slice(2 * lp, 2 * lp + 2)
                            zrhs = (
                                et_t[:, lp, :, :].transpose([0, 2, 1])
                                if swi
                                else et_t[:, lpsl]
                            )
                            nc.tensor.matmul(
                                pz,
                                lhsT=ind_t[:, lpsl, 0:32],
                                rhs=zrhs,
                                start=(lp == 0),
                                stop=(lp == LC // 2 - 1),
                                perf_mode=DR,
                                skip_group_check=True,
                            )
                        zrow = scrp.tile(
                            [1, YG], F32, tag="zrow", name="zrow"
                        )[:, :gw]
                        if swi:
                            # pz columns carry the reversed-ut y order;
                            # un-reverse per 128-subtile during the copy
                            zsrc = pz[0:1].rearrange(
                                "p (s q) -> p s q", q=128
                            )[:, :, ::-1]
                            zdst = zrow.rearrange("p (s q) -> p s q", q=128)
                            nc.vector.tensor_copy(zdst, zsrc)
                        else:
                            nc.vector.tensor_copy(zrow, pz[0:1])

                      for s in range(gw // 128):
                        j = goff // 128 + s
                        ssl = slice(s * 128, (s + 1) * 128)
                        pm = ps2.tile([128, 512], F32, tag="pm")
                        pmr = (
                            psr.tile([128, 512], F32, tag="pmr", name="pmr")
                            if comp_x
                            else None
                        )
                        for lp in range(LC // 2):
                            lpsl = slice(2 * lp, 2 * lp + 2)
                            lhs2 = (
                                et_t[:, lp, ssl, :].rearrange("p y a -> p (y a)")
                                if swi
                                else et_t[:, lpsl, ssl]
                            )
                            last_lp = lp == LC // 2 - 1
                            nc.tensor.matmul(
                                pm,
                                lhsT=lhs2,
                                rhs=xa_t[:, lpsl],
                                start=(lp == 0),
                                stop=last_lp,
                                perf_mode=MAIN_PM,
                                skip_group_check=True,
                            )
                            if comp_x:
                                # same stationary as the main matmul; xr's
                                # col 511 carries the l-validity indicator,
                                # so pmr[:, 511] accumulates Z for free
                                nc.tensor.matmul(
                                    pmr,
                                    lhsT=lhs2,
                                    rhs=xr_t[:, lpsl],
                                    start=(lp == 0),
                                    stop=last_lp,
                                    perf_mode=MAIN_PM,
                                    skip_group_check=True,
                                )
                        if skip_final:
                            continue
                        prod = scrp.tile([128, D], F32, tag="prod")
                        t_acc = smallp.tile([128, 1], F32, tag="t")
                        nc.vector.tensor_tensor(prod, pm, fw_all[:, j], mult)
                        nc.vector.reduce_sum(
                            t_acc, prod, axis=mybir.AxisListType.XYZW
                        )
                        zr = smallp.tile([128, 1], F32, tag="zr")
                        if comp_x:
                            prod2 = scrp.tile([128, D], F32, tag="prod2")
                            t2 = smallp.tile([128, 1], F32, tag="t2")
                            nc.vector.tensor_tensor(
                                prod2, pmr, fwz_all[:, j], mult
                            )
                            nc.vector.reduce_sum(
                                t2, prod2, axis=mybir.AxisListType.XYZW
                            )
                            t_fin = smallp.tile([128, 1], F32, tag="tf")
                            nc.vector.tensor_add(t_fin, t_acc, t2)
                            nc.vector.reciprocal(zr, pmr[:, 511:512])
                        elif z_mode == "mm":
                            t_fin = t_acc
                            zT = psz.tile([128, 4], F32, tag="zT", name="zT")
                            nc.tensor.transpose(
                                zT[:, 0:1], zrow[:, ssl], ident1
                            )
                            nc.vector.reciprocal(zr, zT[:, 0:1])
                        else:
                            t_fin = t_acc
                            nc.vector.reciprocal(zr, pm[:, 0:1])
                        nc.vector.tensor_scalar(
                            out_t[:, b * JTOT + j: b * JTOT + j + 1],
                            t_fin,
                            zr,
                            bias_t[:, j: j + 1],
                            mult,
                            add,
                        )

             nc.sync.dma_start(out_d[:, :], out_t)

    if thin_sems and loop_n == 1:
        _thin_engine_clock_updates(nc)
    _split_sync_waits(nc)
    return nc


_PROGRAM = None


def _get_program():
    global _PROGRAM
    if _PROGRAM is None:
        _PROGRAM = build_program()
    return _PROGRAM


def make_in_maps(x, U_weight, final_weight, final_bias,
                 comp_x="fused", comp_u=None, z_mode="mm", mm_mode="swi"):
    swi = mm_mode == "swi"
    x = np.asarray(x, dtype=np.float32)
    U_weight = np.asarray(U_weight, dtype=np.float32)
    final_weight = np.asarray(final_weight, dtype=np.float32)
    final_bias = np.asarray(final_bias, dtype=np.float32)

    # x: pad L, quantize fp8; natural + transposed layouts (+ residual).
    # SWI mode: MM1's software-interleaved stationary is loaded with its
    # columns (l within a 128-chunk) reversed by the PE, so every l-indexed
    # tensor (xa/xr/ind and the interleaved xt itself) stores l reversed
    # within each chunk; likewise MM2's stationary (eT) gets its columns
    # (y within a 128-subtile) reversed, absorbed by reversing ut/ur
    # y-blocks and un-reversing fw/bias/output on the host.
    xpad = np.zeros((B, LPAD, D), dtype=np.float32)
    xpad[:, :L] = x
    x8 = xpad.astype(nf8)
    x8f = x8.astype(np.float32)
    xrsd = (xpad - x8f).astype(nf8)
    xa_all = np.ascontiguousarray(x8.reshape(B, LC, 128, D))
    if swi:
        # SWI loads reverse the stationary's columns, so the interleaved
        # xt stores l reversed within each 128-chunk; MM1's output rows
        # then come out in natural l order and xa/xr/ind stay natural.
        # xt_il[b, dp, d_p, lc, 2*lr + a]
        #   = x8[b, lc*128 + (127-lr), dp*256 + a*128 + d_p]
        x8rev = np.ascontiguousarray(
            x8.reshape(B, LC, 128, D)[:, :, ::-1, :]
        ).reshape(B, LPAD, D)
        t = x8rev.reshape(B, LC, 128, DC // 2, 2, 128)
        xt_all = np.ascontiguousarray(
            t.transpose(0, 3, 5, 1, 2, 4)
        ).reshape(B, DC // 2, 128, LC, 256)
    else:
        xt_all = np.ascontiguousarray(
            x8.transpose(0, 2, 1)
        ).reshape(B, DC, 128, LPAD)
    xr_all = None
    if comp_x:
        xr_all = xrsd.reshape(B, LC, 128, D).copy()
        # col 511 carries the l-validity indicator so pmr[:, 511] = Z;
        # that column's x-residual compensation is forfeited (1/512 of it)
        lval = (np.arange(LPAD).reshape(LC, 128) < L).astype(nf8)
        xr_all[:, :, :, 511] = lval[None]
        xr_all = np.ascontiguousarray(xr_all)

    YF = YSHARDS * YC
    ufl = np.zeros((YF, D), dtype=np.float32)
    ufl[:Y] = U_weight * SU
    u8 = ufl.astype(nf8)
    u8f = u8.astype(np.float32)
    ursd = (ufl - u8f).astype(nf8)

    def yrev(a):
        # reverse y within each 128-block (rows of a [YC, ...] array):
        # MM2's SWI stationary load reverses eT's columns per subtile, so
        # storing ut/ur with y-blocks pre-reversed makes pm rows natural.
        return np.ascontiguousarray(
            a.reshape(JTOT, 128, *a.shape[1:])[:, ::-1]
        ).reshape(a.shape)

    fwfl = np.zeros((YF, D), dtype=np.float32)
    fwfl[:Y] = final_weight
    bfl = np.zeros((YF,), dtype=np.float32)
    bfl[:Y] = final_bias

    ut_s, ur_s, fw_s, fwz_s, bias_s = [], [], [], [], []
    for ys in range(YSHARDS):
        sl = slice(ys * YC, (ys + 1) * YC)
        u_sh, ur_sh = u8[sl], ursd[sl]
        fw_sh, b_sh = fwfl[sl], bfl[sl]
        if swi:
            u_sh, ur_sh = yrev(u_sh), yrev(ur_sh)
        ut_s.append(np.ascontiguousarray(u_sh.T).reshape(DC, 128, YC))
        if comp_u:
            ur_s.append(np.ascontiguousarray(ur_sh.T).reshape(DC, 128, YC))
        fw_s.append(
            np.ascontiguousarray(fw_sh.astype(nbf16)).reshape(JTOT, 128, D)
        )
        if comp_x:
            fwz = fw_sh.astype(nbf16).copy()
            fwz[:, 511] = 0
            fwz_s.append(np.ascontiguousarray(fwz).reshape(JTOT, 128, D))
        bias_s.append(np.ascontiguousarray(b_sh.reshape(JTOT, 128).T))

    ind = np.zeros((128, LC, 32), dtype=nf8)
    lidx = np.arange(LPAD).reshape(LC, 128)  # [lc, p] -> l
    ind[:, :, 0] = (lidx.T < L).astype(nf8)

    in_maps = []
    for c in range(NCORES):
        ys, bs = c // BSHARDS, c % BSHARDS
        m = {
            "xt": np.ascontiguousarray(xt_all[bs * BC:(bs + 1) * BC]),
            "xa": np.ascontiguousarray(xa_all[bs * BC:(bs + 1) * BC]),
            "ut": ut_s[ys],
            "fw": fw_s[ys],
            "bias": bias_s[ys],
        }
        if comp_x:
            m["xr"] = np.ascontiguousarray(xr_all[bs * BC:(bs + 1) * BC])
            m["fwz"] = fwz_s[ys]
        if comp_u:
            m["ur"] = ur_s[ys]
        if z_mode == "mm" and not comp_x:
            m["ind"] = ind.copy()
        in_maps.append(m)
    return in_maps


def gather_output(results, mm_mode="swi"):
    yfull = np.zeros((B, YSHARDS * YC), dtype=np.float32)
    for c in range(NCORES):
        ys, bs = c // BSHARDS, c % BSHARDS
        o = np.asarray(results[c]["out"], dtype=np.float32)  # (128, BC*JTOT)
        for b in range(BC):
            blk = o[:, b * JTOT:(b + 1) * JTOT]  # (128, 35)
            yfull[bs * BC + b, ys * YC:(ys + 1) * YC] = blk.T.reshape(-1)
    return yfull[:, :Y]


def run(x, U_weight, final_weight, final_bias, **run_kwargs):
    nc = _get_program()
    in_maps = make_in_maps(x, U_weight, final_weight, final_bias)
    res = run_bass_kernel_spmd(nc, in_maps, core_ids=list(range(NCORES)), **run_kwargs)
    return gather_output(res.results), res


def kernel(x, U_weight, final_weight, final_bias):
    out, _ = run(x, U_weight, final_weight, final_bias)
    return out
